# revision 1
# baseline (speedup 1.0000x reference)
"""Trainium2 Bass kernel for nn_LoraQKV (MLA-style LoRA QKV + causal SDPA + o_proj).

Strategy (8 NeuronCores, single NEFF, bf16 matmuls / fp32 PSUM):
  Phase 1 (sequence-sharded): each core computes its 256-token slice of both
    LoRA down-projections, RMSNorm (norm weights folded into the up-proj
    weights host-side), RoPE on the shared k_rope, transposes to
    feature-major bf16 and packs into one [2496, 256] slab.
  AllGather the slabs -> every core holds the full [8*2496, 256] latents.
  Phase 2 (tensor-parallel over heads, 4 heads/core): up-projections emitted
    directly in transposed layout (qT, kT) plus v in natural layout, RoPE on
    q folded into the PSUM->SBUF epilogue, causal attention in scoresT
    layout (exp without max-subtraction -- scores are small by construction;
    row-sums via a ones-column in the PV matmul), then a partial o_proj
    against this core's o_w columns.
  Host sums the 8 partial [2048, 4096] outputs.

Platform workarounds (this walrus build):
  - at most ONE sync-wait per instruction: extra waits are split onto
    standalone EventSemaphore instructions just before lowering.
  - matmul operands must share a partition base, and the base must stay
    constant within a PSUM accumulation group -> everything lives at
    partition base 0; partition shifts go through PSUM-source engine ops
    (free) or SBUF->SBUF DMA.
"""

import os
import sys

sys.path.insert(0, "/opt/trn_rl_repo")

import numpy as np
import ml_dtypes

import bass_rust
import concourse.bass as bass
import concourse.mybir as mybir
import concourse.tile as tile
from concourse.tile import ScopedClock

F32 = mybir.dt.float32
BF16 = mybir.dt.bfloat16

# ---- problem constants (hardcoded per contract) ----
B, S, HID = 1, 2048, 4096
H, HD, ROPE = 32, 128, 64
QR, KVR = 1536, 896
EPS = 1e-6
SCALE = (HD + ROPE) ** -0.5
NCORES = 8
SS = S // NCORES          # 256 tokens per core in phase 1
HPC = H // NCORES         # 4 heads per core in phase 2
PACK = QR + KVR + ROPE    # 2496 rows in the gathered slab
DN = QR + KVR + ROPE      # down-proj output width (q | kv | rope)

# ============================================================
# walrus single-wait workaround
# ============================================================

def _mk_wait(name, engine, wait, debug):
    ev = bass_rust.InstEventSemaphore(name=name, ins=[], outs=[])
    ev.engine = engine
    ev.sync_info = mybir.SyncInfo(on_wait=[wait], on_update=[])
    if debug is not None:
        ev.debug = debug
    return ev


def _split_list(insts):
    out = []
    for inst in insts:
        si = getattr(inst, "sync_info", None)
        ow = list(si.on_wait) if si is not None and si.on_wait else []
        if len(ow) > 1:
            for j, w in enumerate(ow[:-1]):
                out.append(_mk_wait(f"{inst.name}-sw{j}", inst.engine, w,
                                    getattr(inst, "debug", None)))
            inst.sync_info = mybir.SyncInfo(on_wait=[ow[-1]],
                                            on_update=list(si.on_update or []))
        out.append(inst)
    return out


_PATCHED = False


def _install_tile_patches():
    global _PATCHED
    if _PATCHED:
        return
    _PATCHED = True

    _orig_lower = tile.TileContext._lower_ordered_insts

    def _lower_split(self, ordered):
        ordered = {bb: _split_list(insts) for bb, insts in ordered.items()}
        return _orig_lower(self, ordered)

    tile.TileContext._lower_ordered_insts = _lower_split

    def _drain_and_barrier(self, tick_clock, wait_clock):
        nc = self.nc
        probe = nc.sync.nop(nofuse=True)
        wait_clock.add_sem_waits(probe.ins,
                                 ScopedClock({None: tick_clock.global_clock}))
        waits = list(probe.ins.sync_info.on_wait or [])
        probe.ins.sync_info = mybir.SyncInfo(on_wait=waits[:1], on_update=[])
        for w in waits[1:]:
            n = nc.sync.nop(nofuse=True)
            n.ins.sync_info = mybir.SyncInfo(on_wait=[w], on_update=[])
        nc.sync.drain()
        nc.all_engine_barrier()
        assert self.sems is not None
        popped = nc._tile_sem_poison_stack.pop()
        assert popped is self._sem_poison
        nc.clear_and_free_semaphores(list(self.sems.allocated().values()))
        nc.all_engine_barrier()

    tile.TileContext._drain_and_barrier = _drain_and_barrier


# ============================================================
# kernel builder
# ============================================================

def build_nc(debug_phase1=False):
    _install_tile_patches()
    nc = bass.Bass()

    # ---- external inputs (per-core shards prepared host-side) ----
    xT = nc.declare_dram_parameter("xT", [HID // 128, 128, SS], BF16, isOutput=False)
    a_wT = nc.declare_dram_parameter("a_wT", [HID // 128, 128, DN], BF16, isOutput=False)
    cos_sh = nc.declare_dram_parameter("cos_sh", [2, 128, ROPE], F32, isOutput=False)
    sin_sh = nc.declare_dram_parameter("sin_sh", [2, 128, ROPE], F32, isOutput=False)
    qbhiT = nc.declare_dram_parameter("qbhiT", [QR // 128, 128, HPC, 128], BF16, isOutput=False)
    qbloT = nc.declare_dram_parameter("qbloT", [QR // 128, 128, HPC // 2, 128], BF16, isOutput=False)
    kvbkT = nc.declare_dram_parameter("kvbkT", [KVR // 128, 128, HPC, 128], BF16, isOutput=False)
    kvbvT = nc.declare_dram_parameter("kvbvT", [KVR // 128, 128, HPC * 128], BF16, isOutput=False)
    owT = nc.declare_dram_parameter("owT", [HPC, 128, HID], BF16, isOutput=False)
    cosT_s = nc.declare_dram_parameter("cosT_s", [ROPE, S], F32, isOutput=False)   # cos.T * SCALE
    sinTn_s = nc.declare_dram_parameter("sinTn_s", [ROPE, S], F32, isOutput=False) # signed sin.T * SCALE
    ident = nc.declare_dram_parameter("ident", [128, 128], BF16, isOutput=False)
    cmask = nc.declare_dram_parameter("cmask", [4, 128, 512], BF16, isOutput=False)  # -30000 where masked

    o_part = nc.declare_dram_parameter("o_part", [S, HID], BF16, isOutput=True)
    if debug_phase1:
        dbg_gath = nc.declare_dram_parameter("dbg_gath", [NCORES * PACK, SS], F32, isOutput=True)

    KVW = KVR + ROPE  # 960
    pack_kv = nc.dram_tensor("pack_kv", [KVW, SS], BF16)
    pack_q = nc.dram_tensor("pack_q", [QR, SS], BF16)
    gath_kv = nc.dram_tensor("gath_kv", [NCORES * KVW, SS], BF16, addr_space="Shared")
    gath_q = nc.dram_tensor("gath_q", [NCORES * QR, SS], BF16, addr_space="Shared")

    KC = HID // 128  # 32 k-chunks

    with tile.TileContext(nc) as tc:
        # ================= phase 1: down-proj on this core's 256 tokens ========
        with tc.tile_pool(name="p1_const", bufs=1) as cpool, \
             tc.tile_pool(name="p1_w", bufs=3) as wpool, \
             tc.tile_pool(name="p1_out", bufs=1) as opool, \
             tc.tile_pool(name="p1_tmp", bufs=2) as tpool, \
             tc.tile_pool(name="p1_ps", bufs=1, space="PSUM") as psp, \
             tc.tile_pool(name="p1_pst", bufs=2, space="PSUM") as psp_t:
            xT_sb = cpool.tile([128, KC, SS], BF16)
            nc.sync.dma_start(xT_sb[:], xT.ap().rearrange("c p s -> p c s"))
            id_sb = cpool.tile([128, 128], BF16)
            nc.sync.dma_start(id_sb[:], ident[:])
            cos_sb = cpool.tile([128, 2, ROPE], F32)
            nc.sync.dma_start(cos_sb[:], cos_sh.ap().rearrange("b p r -> p b r"))
            sin_sb = cpool.tile([128, 2, ROPE], F32)
            nc.sync.dma_start(sin_sb[:], sin_sh.ap().rearrange("b p r -> p b r"))

            pack_kv_sb = opool.tile([128, 8, SS], BF16)   # 7 kv chunks + rope chunk
            pack_q_sb = opool.tile([128, 12, SS], BF16)

            # -- pass A: kv_lat + rope (cols 1536:2496); 4 psum banks
            pskv = [[psp.tile([128, 512], F32, name=f"pskv{sb}{nt}", tag=f"ps_big_{sb * 2 + nt}") for nt in range(2)]
                    for sb in range(2)]
            for k in range(KC):
                wkv = wpool.tile([128, KVW], BF16, tag="wkv")
                nc.sync.dma_start(wkv[:], a_wT.ap()[k, :, QR:QR + KVW])
                for sb in range(2):
                    nc.tensor.matmul(pskv[sb][0][:], xT_sb[:, k, sb * 128:(sb + 1) * 128],
                                     wkv[:, 0:512], start=(k == 0), stop=(k == KC - 1))
                    nc.tensor.matmul(pskv[sb][1][:, 0:KVW - 512],
                                     xT_sb[:, k, sb * 128:(sb + 1) * 128],
                                     wkv[:, 512:KVW], start=(k == 0), stop=(k == KC - 1))
            for sb in range(2):
                sqt = tpool.tile([128, 512], F32, tag="sqt")
                nt8 = tpool.tile([128, 8], F32, tag="nt8")
                nc.scalar.activation(sqt[:], pskv[sb][0][:],
                                     mybir.ActivationFunctionType.Square,
                                     accum_out=nt8[:, 0:1])
                nc.scalar.activation(sqt[:, 0:KVR - 512], pskv[sb][1][:, 0:KVR - 512],
                                     mybir.ActivationFunctionType.Square,
                                     accum_out=nt8[:, 1:2])
                nc.vector.reduce_sum(nt8[:, 4:5], nt8[:, 0:2], axis=mybir.AxisListType.X)
                nc.vector.tensor_scalar(nt8[:, 5:6], nt8[:, 4:5], 1.0 / KVR, EPS,
                                        mybir.AluOpType.mult, mybir.AluOpType.add)
                nc.scalar.activation(nt8[:, 5:6], nt8[:, 5:6],
                                     mybir.ActivationFunctionType.Sqrt)
                nc.vector.reciprocal(nt8[:, 6:7], nt8[:, 5:6])
                kvn = tpool.tile([128, KVR], BF16, tag="kvn")
                nc.vector.tensor_scalar_mul(kvn[:, 0:512], pskv[sb][0][:], nt8[:, 6:7])
                nc.vector.tensor_scalar_mul(kvn[:, 512:KVR], pskv[sb][1][:, 0:KVR - 512],
                                            nt8[:, 6:7])
                for rc in range(7):
                    pst = psp_t.tile([128, 128], BF16, tag="pst")
                    nc.tensor.transpose(pst[:], kvn[:, rc * 128:(rc + 1) * 128], id_sb[:])
                    nc.scalar.copy(pack_kv_sb[:, rc, sb * 128:(sb + 1) * 128], pst[:])
                # rope on k_rope = pskv[sb][1][:, 384:448]; free-dim rotate, all base 0
                RP = KVR - 512  # 384
                t1 = tpool.tile([128, ROPE], F32, tag="ropet1")
                t2 = tpool.tile([128, ROPE], F32, tag="ropet2")
                nc.vector.tensor_mul(t1[:], pskv[sb][1][:, RP:RP + ROPE], cos_sb[:, sb, :])
                nc.vector.tensor_mul(t2[:, 0:32], pskv[sb][1][:, RP + 32:RP + 64],
                                     sin_sb[:, sb, 0:32])
                nc.vector.tensor_sub(t1[:, 0:32], t1[:, 0:32], t2[:, 0:32])
                nc.vector.tensor_mul(t2[:, 32:64], pskv[sb][1][:, RP:RP + 32],
                                     sin_sb[:, sb, 32:64])
                nc.vector.tensor_add(t1[:, 32:64], t1[:, 32:64], t2[:, 32:64])
                kr = tpool.tile([128, ROPE], BF16, tag="kr")
                nc.vector.tensor_copy(kr[:], t1[:])
                pst = psp_t.tile([128, 128], BF16, tag="pst")
                nc.tensor.transpose(pst[0:ROPE, :], kr[:], id_sb[:])
                nc.scalar.copy(pack_kv_sb[0:ROPE, 7, sb * 128:(sb + 1) * 128], pst[0:ROPE, :])
            nc.gpsimd.dma_start(pack_kv.ap()[0:KVR].rearrange("(c p) s -> p c s", p=128),
                              pack_kv_sb[:, 0:7, :])
            nc.gpsimd.dma_start(pack_kv.ap()[KVR:KVW], pack_kv_sb[0:ROPE, 7, :])
            nc.gpsimd.collective_compute(
                "AllGather", mybir.AluOpType.bypass,
                replica_groups=[list(range(NCORES))],
                ins=[pack_kv.ap().opt()],
                outs=[gath_kv.ap().opt()],
            )

            # -- pass B: q_lat (cols 0:1536), both s-blocks; 6 psum banks
            psq = [[psp.tile([128, 512], F32, name=f"psq{sb}{nt}", tag=f"ps_big_{sb * 3 + nt}") for nt in range(3)]
                   for sb in range(2)]
            for k in range(KC):
                wq = wpool.tile([128, QR], BF16, tag="wq")
                nc.sync.dma_start(wq[:], a_wT.ap()[k, :, 0:QR])
                for sb in range(2):
                    for nt in range(3):
                        nc.tensor.matmul(psq[sb][nt][:],
                                         xT_sb[:, k, sb * 128:(sb + 1) * 128],
                                         wq[:, nt * 512:(nt + 1) * 512],
                                         start=(k == 0), stop=(k == KC - 1))
            for sb in range(2):
                sqt = tpool.tile([128, 512], F32, tag="sqt")
                nt8 = tpool.tile([128, 8], F32, tag="nt8")
                for nt in range(3):
                    nc.scalar.activation(sqt[:], psq[sb][nt][:],
                                         mybir.ActivationFunctionType.Square,
                                         accum_out=nt8[:, nt:nt + 1])
                nc.vector.reduce_sum(nt8[:, 4:5], nt8[:, 0:3], axis=mybir.AxisListType.X)
                nc.vector.tensor_scalar(nt8[:, 5:6], nt8[:, 4:5], 1.0 / QR, EPS,
                                        mybir.AluOpType.mult, mybir.AluOpType.add)
                nc.scalar.activation(nt8[:, 5:6], nt8[:, 5:6],
                                     mybir.ActivationFunctionType.Sqrt)
                nc.vector.reciprocal(nt8[:, 6:7], nt8[:, 5:6])
                qn = tpool.tile([128, QR], BF16, tag="qn")
                for nt in range(3):
                    nc.vector.tensor_scalar_mul(qn[:, nt * 512:(nt + 1) * 512],
                                                psq[sb][nt][:], nt8[:, 6:7])
                for rc in range(12):
                    pst = psp_t.tile([128, 128], BF16, tag="pst")
                    nc.tensor.transpose(pst[:], qn[:, rc * 128:(rc + 1) * 128], id_sb[:])
                    nc.scalar.copy(pack_q_sb[:, rc, sb * 128:(sb + 1) * 128], pst[:])
            nc.sync.dma_start(pack_q.ap().rearrange("(c p) s -> p c s", p=128),
                              pack_q_sb[:, 0:12, :])



        if debug_phase1:
            with tc.tile_pool(name="dbg", bufs=2) as dpool:
                for blk in range(NCORES * QR // 128):
                    gb = dpool.tile([128, SS], BF16, tag="gb")
                    nc.sync.dma_start(gb[:], gath_q.ap()[blk * 128:(blk + 1) * 128, :])
                    gf = dpool.tile([128, SS], F32, tag="gf")
                    nc.vector.tensor_copy(gf[:], gb[:])
                    nc.sync.dma_start(dbg_gath.ap()[blk * 128:(blk + 1) * 128, :], gf[:])

        # ================= phase 2 =================
        QB = QR // 128    # 12
        KB = KVR // 128   # 7
        with tc.tile_pool(name="p2_const", bufs=1) as wp2, \
             tc.tile_pool(name="p2_qkv", bufs=1) as qkvp, \
             tc.tile_pool(name="p2_tmp", bufs=2) as tp2:
            id2_sb = wp2.tile([128, 128], BF16)
            nc.sync.dma_start(id2_sb[:], ident[:])
            cmask_sb = wp2.tile([128, 4, 512], BF16)
            nc.sync.dma_start(cmask_sb[:], cmask.ap().rearrange("m p c -> p m c"))

            ropeT = qkvp.tile([ROPE, S], BF16)
            q_rope = [qkvp.tile([ROPE, 4, 512], BF16, name=f"q_rope_{h}") for h in range(HPC)]
            q_nope = [qkvp.tile([128, 4, 512], BF16, name=f"q_nope_{h}") for h in range(HPC)]
            k_nope = [qkvp.tile([128, 16, 128], BF16, name=f"k_nope_{h}") for h in range(HPC)]
            v_sb = qkvp.tile([128, 16, HPC, 192], BF16)
            attnT = qkvp.tile([128, HPC, 16, 128], BF16)

            # ---- kv up-proj (overlaps q AllGather) ----
            with tc.tile_pool(name="p2_latkv", bufs=1) as latkv, \
                 tc.tile_pool(name="p2_pskv", bufs=1, space="PSUM") as pskvp:
                kvlatT = latkv.tile([128, KB, S], BF16)
                for b in range(NCORES):
                    base = b * KVW
                    nc.gpsimd.dma_start(
                        kvlatT[:, :, b * SS:(b + 1) * SS],
                        gath_kv.ap()[base:base + KVR].rearrange("(c p) s -> p c s", p=128))
                    nc.gpsimd.dma_start(ropeT[:, b * SS:(b + 1) * SS],
                                      gath_kv.ap()[base + KVR:base + KVW])
                nc.gpsimd.collective_compute(
                    "AllGather", mybir.AluOpType.bypass,
                    replica_groups=[list(range(NCORES))],
                    ins=[pack_q.ap().opt()],
                    outs=[gath_q.ap().opt()],
                )
                kvbk_sb = latkv.tile([128, KB, HPC, 128], BF16)
                nc.sync.dma_start(kvbk_sb[:], kvbkT.ap().rearrange("c p h d -> p c h d"))
                kvbv_sb = latkv.tile([128, KB, HPC * 128], BF16)
                nc.sync.dma_start(kvbv_sb[:], kvbvT.ap().rearrange("c p d -> p c d"))
                for h in range(HPC):
                    for st in range(4):
                        ps_k = pskvp.tile([128, 512], F32, tag="ps_k")
                        for rc in range(KB):
                            nc.tensor.matmul(ps_k[:], kvbk_sb[:, rc, h, :],
                                             kvlatT[:, rc, st * 512:(st + 1) * 512],
                                             start=(rc == 0), stop=(rc == KB - 1))
                        nc.vector.tensor_copy(k_nope[h][:, st * 4:(st + 1) * 4, :], ps_k[:])
                nc.vector.memset(v_sb[:, :, :, 64:65], 1.0)
                nc.vector.memset(v_sb[:, :, :, 65:128], 0.0)
                for sk in range(16):
                    ps_v = pskvp.tile([128, 512], F32, tag="ps_v")
                    for rc in range(KB):
                        nc.tensor.matmul(ps_v[:], kvlatT[:, rc, sk * 128:(sk + 1) * 128],
                                         kvbv_sb[:, rc, :],
                                         start=(rc == 0), stop=(rc == KB - 1))
                    for h in range(HPC):
                        nc.scalar.copy(v_sb[:, sk, h, 0:64], ps_v[:, h * 128:h * 128 + 64])
                        nc.scalar.copy(v_sb[:, sk, h, 128:192], ps_v[:, h * 128 + 64:(h + 1) * 128])

            # ---- q up-proj: resident qlatT ----
            with tc.tile_pool(name="p2_latq", bufs=1) as latq, \
                 tc.tile_pool(name="p2_psup", bufs=1, space="PSUM") as psup:
                qbhi_sb = latq.tile([128, QB, HPC, 128], BF16)
                nc.sync.dma_start(qbhi_sb[:], qbhiT.ap().rearrange("c p h d -> p c h d"))
                qblo_sb = latq.tile([128, QB, HPC // 2, 128], BF16)
                nc.sync.dma_start(qblo_sb[:], qbloT.ap().rearrange("c p h d -> p c h d"))
                cosT_sb = latq.tile([ROPE, S], F32)
                nc.sync.dma_start(cosT_sb[:], cosT_s[:])
                sinT_sb = latq.tile([ROPE, S], F32)
                nc.sync.dma_start(sinT_sb[:], sinTn_s[:])
                qlat_ch = [latq.tile([128, S], BF16, name=f"qlat{rc}")
                           for rc in range(QB)]
                for rc in range(QB):
                    for b in range(NCORES):
                        nc.sync.dma_start(
                            qlat_ch[rc][:, b * SS:(b + 1) * SS],
                            gath_q.ap()[b * QR + rc * 128:b * QR + (rc + 1) * 128, :])
                for qt in range(4):
                    ps_hi = [psup.tile([128, 512], F32, name=f"ps_hi{h}", tag=f"ps_hi{h}")
                             for h in range(HPC)]
                    ps_lo = [psup.tile([128, 512], F32, name=f"ps_lo{p}", tag=f"ps_lo{p}")
                             for p in range(HPC // 2)]
                    qsl = slice(qt * 512, (qt + 1) * 512)
                    for rc in range(QB):
                        st, sp = (rc == 0), (rc == QB - 1)
                        for h in range(HPC):
                            nc.tensor.matmul(ps_hi[h][:], qbhi_sb[:, rc, h, :],
                                             qlat_ch[rc][:, qsl], start=st, stop=sp)
                        for p in range(HPC // 2):
                            nc.tensor.matmul(ps_lo[p][:], qblo_sb[:, rc, p, :],
                                             qlat_ch[rc][:, qsl], start=st, stop=sp)
                    for h in range(HPC):
                        pr, i = h // 2, h % 2
                        nc.scalar.activation(q_nope[h][0:64, qt, :], ps_hi[h][64:128, :],
                                             mybir.ActivationFunctionType.Copy, scale=SCALE)
                        nc.scalar.activation(q_nope[h][64:128, qt, :],
                                             ps_lo[pr][i * 64:(i + 1) * 64, :],
                                             mybir.ActivationFunctionType.Copy, scale=SCALE)
                        stg = tp2.tile([ROPE, 512], F32, tag="stg")
                        nc.scalar.copy(stg[:], ps_hi[h][0:ROPE, :])
                        rot = tp2.tile([ROPE, 512], F32, tag="rot")
                        nc.gpsimd.dma_start(rot[0:32, :], stg[32:64, :])
                        nc.gpsimd.dma_start(rot[32:64, :], stg[0:32, :])
                        m1 = tp2.tile([ROPE, 512], F32, tag="m1")
                        nc.vector.tensor_mul(m1[:], stg[:], cosT_sb[:, qsl])
                        m2 = tp2.tile([ROPE, 512], F32, tag="m2")
                        nc.vector.tensor_mul(m2[:], rot[:], sinT_sb[:, qsl])
                        nc.vector.tensor_add(q_rope[h][:, qt, :], m1[:], m2[:])

            # ---- o_proj weights loaded early to overlap attention ----
            _ow_ctx = tc.tile_pool(name="p2_ow", bufs=1)
            owp = _ow_ctx.__enter__()
            owT_sb = owp.tile([128, HPC, HID], BF16)
            nc.sync.dma_start(owT_sb[:], owT.ap().rearrange("h p d -> p h d"))

            # ---- causal attention (J-outer) with interleaved o_proj ----
            with tc.tile_pool(name="p2_psatt", bufs=3, space="PSUM") as psat, \
                 tc.tile_pool(name="p2_pso", bufs=2, space="PSUM") as psop, \
                 tc.tile_pool(name="p2_pso2", bufs=1, space="PSUM") as psop2, \
                 tc.tile_pool(name="p2_exp", bufs=4) as expp, \
                 tc.tile_pool(name="p2_psoo", bufs=2, space="PSUM") as psoop, \
                 tc.tile_pool(name="p2_dr", bufs=4, space="DRAM") as drp:
                def emit_oproj(sblk, ot):
                    ps_oo = psoop.tile([128, 512], F32, tag="ps_oo")
                    for hh in range(HPC):
                        nc.tensor.matmul(ps_oo[:], attnT[:, hh, sblk, :],
                                         owT_sb[:, hh, ot * 512:(ot + 1) * 512],
                                         start=(hh == 0), stop=(hh == HPC - 1))
                    oo = tp2.tile([128, 512], BF16, tag="oo")
                    nc.vector.tensor_copy(oo[:], ps_oo[:])
                    nc.sync.dma_start(
                        o_part.ap()[sblk * 128:(sblk + 1) * 128, ot * 512:(ot + 1) * 512],
                        oo[:])

                pending = []
                for J in (3, 2, 1, 0):
                    nsk = 4 * J + 4
                    for h in range(HPC):
                        ps_A = psop.tile([128, 512], F32, name="ps_A", tag="ps_A")
                        ps_B = psop2.tile([64, 512], F32, name="ps_B", tag="ps_B")
                        prev_expT = None
                        for b in range(nsk):
                            diag = b >= 4 * J
                            m = b - 4 * J if diag else 0
                            col0 = 128 * m if diag else 0
                            ps_s = psat.tile([128, 512], F32, tag="ps_s")
                            if diag:
                                nc.tensor.matmul(ps_s[:], id2_sb[:],
                                                 cmask_sb[:, m, :],
                                                 start=True, stop=False)
                            nc.tensor.matmul(ps_s[:, col0:512],
                                             ropeT[:, b * 128:(b + 1) * 128],
                                             q_rope[h][:, J, col0:512],
                                             start=(not diag), stop=False)
                            nc.tensor.matmul(ps_s[:, col0:512],
                                             k_nope[h][:, b, :],
                                             q_nope[h][:, J, col0:512],
                                             start=False, stop=True)
                            if prev_expT is not None:
                                pb = b - 1
                                nc.tensor.matmul(ps_A[:], v_sb[:, pb, h, 0:128],
                                                 prev_expT[:],
                                                 start=(pb == 0), stop=False)
                                nc.tensor.matmul(ps_B[:], v_sb[:, pb, h, 128:192],
                                                 prev_expT[:],
                                                 start=(pb == 0), stop=False)
                            expT = expp.tile([128, 512], BF16, tag="expT")
                            nc.scalar.activation(expT[:], ps_s[:],
                                                 mybir.ActivationFunctionType.Exp)
                            prev_expT = expT
                        pb = nsk - 1
                        nc.tensor.matmul(ps_A[:], v_sb[:, pb, h, 0:128], prev_expT[:],
                                         start=(pb == 0), stop=True)
                        nc.tensor.matmul(ps_B[:], v_sb[:, pb, h, 128:192], prev_expT[:],
                                         start=(pb == 0), stop=True)
                        rcp = tp2.tile([1, 512], F32, tag="rcp")
                        nc.vector.reciprocal(rcp[:], ps_A[64:65, :])
                        rcd = drp.tile([1, 512], F32, tag="rcd")
                        nc.gpsimd.dma_start(rcd[:], rcp[:])
                        rcf = tp2.tile([128, 512], F32, tag="rcf")
                        nc.gpsimd.dma_start(rcf[:], rcd[0:1].to_broadcast([128, 512]))
                        nc.vector.tensor_mul(attnT[0:64, h, J * 4:(J + 1) * 4, :],
                                             ps_A[0:64, :], rcf[0:64, :])
                        nc.vector.tensor_mul(attnT[64:128, h, J * 4:(J + 1) * 4, :],
                                             ps_B[:], rcf[64:128, :])
                        # interleave previous-J o_proj chunks
                        for _ in range(8):
                            if pending:
                                emit_oproj(*pending.pop(0))
                    pending.extend([(sblk, ot) for sblk in range(4 * J, 4 * J + 4)
                                    for ot in range(8)])
                while pending:
                    emit_oproj(*pending.pop(0))
            _ow_ctx.__exit__(None, None, None)

    return nc


# ============================================================
# host-side wrapper
# ============================================================

_BUILT = {}


def _get_nc(debug_phase1=False):
    key = bool(debug_phase1)
    if key not in _BUILT:
        _BUILT[key] = build_nc(debug_phase1)
    return _BUILT[key]


def _bf(x):
    return np.ascontiguousarray(x).astype(ml_dtypes.bfloat16)


def prepare_in_maps(hidden_states, cos, sin, q_a_w, q_a_norm_w, q_b_w,
                    kv_a_w, kv_a_norm_w, kv_b_w, o_w):
    hidden_states = np.asarray(hidden_states, dtype=np.float32)
    cos = np.asarray(cos, dtype=np.float32)
    sin = np.asarray(sin, dtype=np.float32)
    q_a_w = np.asarray(q_a_w, dtype=np.float32)
    q_a_norm_w = np.asarray(q_a_norm_w, dtype=np.float32)
    q_b_w = np.asarray(q_b_w, dtype=np.float32)
    kv_a_w = np.asarray(kv_a_w, dtype=np.float32)
    kv_a_norm_w = np.asarray(kv_a_norm_w, dtype=np.float32)
    kv_b_w = np.asarray(kv_b_w, dtype=np.float32)
    o_w = np.asarray(o_w, dtype=np.float32)

    x = hidden_states.reshape(S, HID)
    a_w = np.concatenate([q_a_w, kv_a_w], axis=0)          # [QR + KVR + ROPE, HID]
    a_wT = _bf(a_w.T).reshape(HID // 128, 128, DN)
    qb = q_b_w * q_a_norm_w[None, :]                       # [H*(ROPE+HD), QR]
    qbTh = qb.reshape(H, ROPE + HD, QR)
    kvb = kv_b_w * kv_a_norm_w[None, :]                    # [H*2*HD, KVR]
    kvbTh = kvb.reshape(H, 2 * HD, KVR)

    cosT_arr = np.ascontiguousarray(cos.T * SCALE).astype(np.float32)      # [64, S]
    sinT = sin.T * SCALE
    sinTn_arr = np.concatenate([-sinT[0:32], sinT[32:64]], axis=0).astype(np.float32)
    identity = np.eye(128, dtype=np.float32).astype(ml_dtypes.bfloat16)
    cm = np.zeros((4, 128, 512), np.float32)
    for mm_ in range(4):
        p = np.arange(128)[:, None]
        c = np.arange(512)[None, :]
        cm[mm_] = np.where(p + 128 * mm_ <= c, 0.0, -30000.0)
    cmask_arr = cm.astype(ml_dtypes.bfloat16)

    in_maps = []
    for c in range(NCORES):
        sl = slice(c * SS, (c + 1) * SS)
        xT_c = _bf(x[sl].T).reshape(HID // 128, 128, SS)
        cos_c = np.ascontiguousarray(cos[sl]).reshape(2, 128, ROPE)
        sin_c = np.ascontiguousarray(sin[sl]).reshape(2, 128, ROPE)
        hsl = slice(c * HPC, (c + 1) * HPC)
        qbh = qbTh[hsl]                                  # [4, 192, QR]
        qbhiT_c = _bf(qbh[:, 0:128, :].transpose(2, 0, 1)).reshape(QR // 128, 128, HPC, 128)
        # lo: pack pairs (hA lo 64 | hB lo 64) contiguously
        qblo = qbh[:, 128:192, :].reshape(HPC // 2, 128, QR)  # [2, 2*64, QR]
        qbloT_c = _bf(qblo.transpose(2, 0, 1)).reshape(QR // 128, 128, HPC // 2, 128)
        kvbh = kvbTh[hsl]                                # [4, 256, KVR]
        kvbkT_c = _bf(kvbh[:, 0:128, :].transpose(2, 0, 1)).reshape(KVR // 128, 128, HPC, 128)
        kvbvT_c = _bf(kvbh[:, 128:256, :].reshape(HPC * 128, KVR).T).reshape(KVR // 128, 128, HPC * 128)
        owT_c = _bf(o_w[:, c * HPC * HD:(c + 1) * HPC * HD].T).reshape(HPC, 128, HID)
        in_maps.append({
            "xT": xT_c, "a_wT": a_wT, "cos_sh": cos_c, "sin_sh": sin_c,
            "qbhiT": qbhiT_c, "qbloT": qbloT_c, "kvbkT": kvbkT_c,
            "kvbvT": kvbvT_c, "owT": owT_c,
            "cosT_s": cosT_arr, "sinTn_s": sinTn_arr,
            "ident": identity, "cmask": cmask_arr,
        })
    return in_maps


def run_on_cores(in_maps, debug_phase1=False, trace=False):
    from concourse.bass_utils import run_bass_kernel_spmd
    nc = _get_nc(debug_phase1)
    return run_bass_kernel_spmd(nc, in_maps, core_ids=list(range(NCORES)), trace=trace)


def kernel(**inputs):
    in_maps = prepare_in_maps(**inputs)
    res = run_on_cores(in_maps)
    out = np.zeros((S, HID), np.float64)
    for c in range(NCORES):
        out += res.results[c]["o_part"].astype(np.float64)
    return out.astype(np.float32).reshape(B, S, HID)

